# revision 7
# baseline (speedup 1.0000x reference)
"""DeltaRuleMemory Trainium2 kernel: 8-way sequence-sharded chunkwise WY form.

Math (W = M^T, a = LR/B):
  per step: e = v - kW ; W += a k^T e ; y = qW
Chunkwise over R=128 rows (16 steps x batch 8):
  A_low = I + a*strict_blocklower(K K^T);  E = A_low^{-1} (V - K W_in)
  W_out = W_in + a K^T E;   Y = Q W_in + a inclmask(Q K^T) E
Cross-core (sequence split 8 ways): phase 1 computes per-core affine
transition W_end = P W_start + G via the augmented chain X=[P|G];
host composes the 8 transitions to get per-core W_start; phase 2
replays chunks computing E and Y with the exact W chain.
T = a*(A_up)^{-1} (A_up = I + a*strict_blockupper(S), S = K K^T) is computed
in phase 1 by nilpotent Neumann factorization (N^16 = 0):
  A_up^{-1} = (I - Nu)(I + Nu^2)(I + Nu^4)(I + Nu^8)
and cached to DRAM for phase 2 reuse.
"""
import numpy as np
from concourse import bass, bacc, tile
from concourse.bass_utils import run_bass_kernel_spmd

mybir = bass.mybir
F32 = mybir.dt.float32
F32R = mybir.dt.float32r
ADD = mybir.AluOpType.add
SUB = mybir.AluOpType.subtract
MUL = mybir.AluOpType.mult

B, S, DK, DV = 8, 4096, 256, 256
NCORE = 8
A = np.float32(0.1 / B)
R = 128                      # rows per chunk


def _host_consts():
    ii = np.arange(R) // B
    strict_u = (ii[:, None] < ii[None, :]).astype(np.float32)
    consts = {
        "amu": A * strict_u,                                   # a*strict upper
        "aml": A * strict_u.T,                                 # a*strict lower
        "incl": (ii[:, None] <= ii[None, :]).astype(np.float32),
        "ident": np.eye(R, dtype=np.float32),
    }
    return consts


def _tinv(nc, pool, ppool, kt, ident_s, amu_s, aml_s, c):
    """Emit T_c = a*(A_up)^{-1} from S=K K^T. Returns f32r SBUF tile [128,128]."""
    ps_s = ppool.tile([R, R], F32, tag="ps_tp", name=f"ps_s{c}")
    nc.tensor.matmul(ps_s[:], kt[0][:], kt[0][:], start=True, stop=False)
    nc.tensor.matmul(ps_s[:], kt[1][:], kt[1][:], start=False, stop=True)
    nu = pool.tile([R, R], F32, tag="nu", name=f"nu{c}")
    nl = pool.tile([R, R], F32, tag="nl", name=f"nl{c}")
    nc.vector.tensor_tensor(out=nu[:], in0=ps_s[:], in1=amu_s[:], op=MUL)
    nc.vector.tensor_tensor(out=nl[:], in0=ps_s[:], in1=aml_s[:], op=MUL)

    def sq(lt, rt, tag):
        ps = ppool.tile([R, R], F32, tag="ps_t", name=f"ps_{tag}{c}")
        nc.tensor.matmul(ps[:], lt[:], rt[:], start=True, stop=True)
        t = pool.tile([R, R], F32, tag=tag, name=f"{tag}{c}")
        nc.any.tensor_copy(out=t[:], in_=ps[:])
        return t

    n2u = sq(nl, nu, "n2u")
    n2l = sq(nu, nl, "n2l")
    n4u = sq(n2l, n2u, "n4u")
    n4l = sq(n2u, n2l, "n4l")
    # T1u = I + N8u (N8u = N4l.T @ N4u stays in psum)
    ps8 = ppool.tile([R, R], F32, tag="ps_t", name=f"ps_n8u{c}")
    nc.tensor.matmul(ps8[:], n4l[:], n4u[:], start=True, stop=True)
    t1u = pool.tile([R, R], F32, tag="t1u", name=f"t1u{c}")
    nc.vector.tensor_tensor(out=t1u[:], in0=ps8[:], in1=ident_s[:], op=ADD)
    # T2u = T1u + N4u T1u
    ps2 = ppool.tile([R, R], F32, tag="ps_t", name=f"ps_t2{c}")
    nc.tensor.matmul(ps2[:], n4l[:], t1u[:], start=True, stop=True)
    t2u = pool.tile([R, R], F32, tag="t2u", name=f"t2u{c}")
    nc.vector.tensor_tensor(out=t2u[:], in0=ps2[:], in1=t1u[:], op=ADD)
    # T3u = T2u + N2u T2u
    ps3 = ppool.tile([R, R], F32, tag="ps_t", name=f"ps_t3{c}")
    nc.tensor.matmul(ps3[:], n2l[:], t2u[:], start=True, stop=True)
    t3u = pool.tile([R, R], F32, tag="t3u", name=f"t3u{c}")
    nc.vector.tensor_tensor(out=t3u[:], in0=ps3[:], in1=t2u[:], op=ADD)
    # Ta = a*(T3u - Nu T3u)
    ps4 = ppool.tile([R, R], F32, tag="ps_t", name=f"ps_t4{c}")
    nc.tensor.matmul(ps4[:], nl[:], t3u[:], start=True, stop=True)
    tmp = pool.tile([R, R], F32, tag="ttmp", name=f"ttmp{c}")
    nc.any.tensor_tensor(out=tmp[:], in0=t3u[:], in1=ps4[:], op=SUB)
    ta = pool.tile([R, R], F32, tag="ta", name=f"ta{c}")
    nc.vector.tensor_scalar_mul(out=ta[:], in0=tmp[:], scalar1=float(A))
    return ta


def _load_kt(nc, pool, ppool, kr, ident_s, c, pfx):
    """PE-transpose both 128-halves of kr [128,256] -> f32r tiles [128,128]x2."""
    kt = []
    for d in range(2):
        pst = ppool.tile([R, R], F32, tag="ps_tp", name=f"ps_{pfx}t{d}_{c}")
        nc.tensor.transpose(pst[:], kr[:, d * R:(d + 1) * R], ident_s[:])
        t = pool.tile([R, R], F32, tag=f"{pfx}t{d}", name=f"{pfx}t{d}_{c}")
        nc.any.tensor_copy(out=t[:], in_=pst[:])
        kt.append(t)
    return kt


def build_phase1(nch, ncore):
    nc = bacc.Bacc("TRN2", target_bir_lowering=False, debug=False,
                   num_devices=ncore)
    rows = nch * R
    k_in = nc.dram_tensor("k", [rows, DK], F32, kind="ExternalInput")
    v_in = nc.dram_tensor("v", [rows, DV], F32, kind="ExternalInput")
    amu_in = nc.dram_tensor("amu", [R, R], F32, kind="ExternalInput")
    aml_in = nc.dram_tensor("aml", [R, R], F32, kind="ExternalInput")
    id_in = nc.dram_tensor("ident", [R, R], F32, kind="ExternalInput")
    x_in = nc.dram_tensor("xinit", [DK, DK + DV], F32, kind="ExternalInput")
    ta_out = nc.dram_tensor("ta_out", [rows, R], F32, kind="ExternalOutput")
    pg_out = nc.dram_tensor("pg", [DK, DK + DV], F32, kind="ExternalOutput")

    with tile.TileContext(nc) as tc:
        with tc.tile_pool(name="state", bufs=1) as spool, \
             tc.tile_pool(name="sbuf", bufs=3) as pool, \
             tc.tile_pool(name="psum", bufs=2, space="PSUM") as ppool, \
             tc.tile_pool(name="psumw", bufs=2, space="PSUM") as ppw:
            amu_s = spool.tile([R, R], F32, name="amu_s")
            aml_s = spool.tile([R, R], F32, name="aml_s")
            ident_s = spool.tile([R, R], F32, name="ident_s")
            nc.sync.dma_start(amu_s[:], amu_in[:, :])
            nc.sync.dma_start(aml_s[:], aml_in[:, :])
            nc.sync.dma_start(ident_s[:], id_in[:, :])
            x = []
            for d in range(2):
                xd = spool.tile([R, DK + DV], F32, name=f"x{d}")
                nc.sync.dma_start(xd[:], x_in[d * R:(d + 1) * R, :])
                x.append(xd)

            for c in range(nch):
                kr = pool.tile([R, DK], F32, tag="kr", name=f"kr{c}")
                vr = pool.tile([R, DV], F32, tag="vr", name=f"vr{c}")
                nc.sync.dma_start(kr[:], k_in[c * R:(c + 1) * R, :])
                nc.sync.dma_start(vr[:], v_in[c * R:(c + 1) * R, :])
                kt = _load_kt(nc, pool, ppool, kr, ident_s, c, "k")
                ta = _tinv(nc, pool, ppool, kt, ident_s, amu_s, aml_s, c)
                nc.sync.dma_start(ta_out[c * R:(c + 1) * R, :], ta[:])

                # H = K X - [0|V]
                ps_h = ppw.tile([R, DK + DV], F32, tag="ps_w", name=f"ps_h{c}")
                nc.tensor.matmul(ps_h[:], kt[0][:], x[0][:], start=True, stop=False)
                nc.tensor.matmul(ps_h[:], kt[1][:], x[1][:], start=False, stop=True)
                hs = pool.tile([R, DK + DV], F32, tag="hs", name=f"hs{c}")
                nc.any.tensor_copy(out=hs[:, 0:DK], in_=ps_h[:, 0:DK])
                nc.vector.tensor_tensor(out=hs[:, DK:], in0=ps_h[:, DK:],
                                        in1=vr[:], op=SUB)
                # J = Ta_low H  (lhsT = Ta upper)
                ps_j = ppw.tile([R, DK + DV], F32, tag="ps_w", name=f"ps_j{c}")
                nc.tensor.matmul(ps_j[:], ta[:], hs[:], start=True, stop=True)
                js = pool.tile([R, DK + DV], F32, tag="js", name=f"js{c}")
                nc.any.tensor_copy(out=js[:], in_=ps_j[:])
                # X -= K^T J
                for d in range(2):
                    ps_x = ppw.tile([R, DK + DV], F32, tag="ps_w",
                                    name=f"ps_x{d}_{c}")
                    nc.tensor.matmul(ps_x[:], kr[:, d * R:(d + 1) * R],
                                     js[:], start=True, stop=True)
                    nc.vector.tensor_tensor(out=x[d][:], in0=x[d][:],
                                            in1=ps_x[:], op=SUB)

            for d in range(2):
                nc.sync.dma_start(pg_out[d * R:(d + 1) * R, :], x[d][:])
    nc.compile()
    return nc


def build_phase2(nch, ncore):
    nc = bacc.Bacc("TRN2", target_bir_lowering=False, debug=False,
                   num_devices=ncore)
    rows = nch * R
    k_in = nc.dram_tensor("k", [rows, DK], F32, kind="ExternalInput")
    v_in = nc.dram_tensor("v", [rows, DV], F32, kind="ExternalInput")
    q_in = nc.dram_tensor("q", [rows, DK], F32, kind="ExternalInput")
    ta_in = nc.dram_tensor("ta", [rows, R], F32, kind="ExternalInput")
    incl_in = nc.dram_tensor("incl", [R, R], F32, kind="ExternalInput")
    id_in = nc.dram_tensor("ident", [R, R], F32, kind="ExternalInput")
    w_in = nc.dram_tensor("winit", [DK, DV], F32, kind="ExternalInput")
    y_out = nc.dram_tensor("y", [rows, DV], F32, kind="ExternalOutput")

    with tile.TileContext(nc) as tc:
        with tc.tile_pool(name="state", bufs=1) as spool, \
             tc.tile_pool(name="sbuf", bufs=3) as pool, \
             tc.tile_pool(name="psum", bufs=2, space="PSUM") as ppool, \
             tc.tile_pool(name="psumw", bufs=2, space="PSUM") as ppw:
            incl_s = spool.tile([R, R], F32, name="incl_s")
            ident_s = spool.tile([R, R], F32, name="ident_s")
            nc.sync.dma_start(incl_s[:], incl_in[:, :])
            nc.sync.dma_start(ident_s[:], id_in[:, :])
            w = []
            for d in range(2):
                wd = spool.tile([R, DV], F32, name=f"w{d}")
                nc.sync.dma_start(wd[:], w_in[d * R:(d + 1) * R, :])
                w.append(wd)

            for c in range(nch):
                kr = pool.tile([R, DK], F32, tag="kr", name=f"kr{c}")
                vr = pool.tile([R, DV], F32, tag="vr", name=f"vr{c}")
                qr = pool.tile([R, DK], F32, tag="qr", name=f"qr{c}")
                tar = pool.tile([R, R], F32, tag="tar", name=f"tar{c}")  # fp32 lhsT
                nc.sync.dma_start(kr[:], k_in[c * R:(c + 1) * R, :])
                nc.sync.dma_start(vr[:], v_in[c * R:(c + 1) * R, :])
                nc.sync.dma_start(qr[:], q_in[c * R:(c + 1) * R, :])
                nc.sync.dma_start(tar[:], ta_in[c * R:(c + 1) * R, :])
                ta = tar
                kt = _load_kt(nc, pool, ppool, kr, ident_s, c, "k")
                qt = _load_kt(nc, pool, ppool, qr, ident_s, c, "q")
                # U = V - K W_in
                ps_u = ppw.tile([R, DV], F32, tag="ps_w", name=f"ps_u{c}")
                nc.tensor.matmul(ps_u[:], kt[0][:], w[0][:], start=True, stop=False)
                nc.tensor.matmul(ps_u[:], kt[1][:], w[1][:], start=False, stop=True)
                us = pool.tile([R, DV], F32, tag="us", name=f"us{c}")
                nc.vector.tensor_tensor(out=us[:], in0=vr[:], in1=ps_u[:], op=SUB)
                # Ehat = a T_low U   (lhsT = Ta)
                ps_e = ppw.tile([R, DV], F32, tag="ps_w", name=f"ps_e{c}")
                nc.tensor.matmul(ps_e[:], ta[:], us[:], start=True, stop=True)
                es = pool.tile([R, DV], F32, tag="es", name=f"es{c}")
                nc.any.tensor_copy(out=es[:], in_=ps_e[:])
                # masked Sqk^T = incl_upper o (K Q^T)
                ps_q = ppool.tile([R, R], F32, tag="ps_tp", name=f"ps_q{c}")
                nc.tensor.matmul(ps_q[:], kt[0][:], qt[0][:], start=True, stop=False)
                nc.tensor.matmul(ps_q[:], kt[1][:], qt[1][:], start=False, stop=True)
                sm = pool.tile([R, R], F32, tag="sm", name=f"sm{c}")
                nc.vector.tensor_tensor(out=sm[:], in0=ps_q[:], in1=incl_s[:], op=MUL)
                # Y = Q W_in + (masked Sqk^T)^T Ehat
                ps_y = ppw.tile([R, DV], F32, tag="ps_w", name=f"ps_y{c}")
                nc.tensor.matmul(ps_y[:], qt[0][:], w[0][:], start=True, stop=False)
                nc.tensor.matmul(ps_y[:], qt[1][:], w[1][:], start=False, stop=False)
                nc.tensor.matmul(ps_y[:], sm[:], es[:], start=False, stop=True)
                ys = pool.tile([R, DV], F32, tag="ys", name=f"ys{c}")
                nc.any.tensor_copy(out=ys[:], in_=ps_y[:])
                nc.sync.dma_start(y_out[c * R:(c + 1) * R, :], ys[:])
                # W += K^T Ehat
                for d in range(2):
                    ps_w = ppool.tile([R, DV], F32, tag="ps_tp",
                                      name=f"ps_w{d}_{c}")
                    nc.tensor.matmul(ps_w[:], kr[:, d * R:(d + 1) * R],
                                     es[:], start=True, stop=True)
                    nc.vector.tensor_tensor(out=w[d][:], in0=w[d][:],
                                            in1=ps_w[:], op=ADD)
    nc.compile()
    return nc


_BUILT = {}
LAST_RESULTS = (None, None)


def _get(phase, nch, ncore):
    key = (phase, nch, ncore)
    if key not in _BUILT:
        _BUILT[key] = (build_phase1 if phase == 1 else build_phase2)(nch, ncore)
    return _BUILT[key]


def kernel(keys, values, queries, memory):
    keys = np.asarray(keys, np.float32)
    values = np.asarray(values, np.float32)
    queries = np.asarray(queries, np.float32)
    memory = np.asarray(memory, np.float32)
    Bs, Ss = keys.shape[0], keys.shape[1]
    spc = Ss // NCORE                      # steps per core
    rows = spc * Bs
    nch = rows // R
    consts = _host_consts()
    W0 = memory.T.astype(np.float32).copy()    # (DK, DV)

    def core_rows(x, i):
        return np.ascontiguousarray(
            x[:, i * spc:(i + 1) * spc, :].transpose(1, 0, 2).reshape(rows, -1))

    Ks = [core_rows(keys, i) for i in range(NCORE)]
    Vs = [core_rows(values, i) for i in range(NCORE)]
    Qs = [core_rows(queries, i) for i in range(NCORE)]

    xinit = np.concatenate([np.eye(DK, dtype=np.float32), W0], axis=1)
    nc1 = _get(1, nch, NCORE)
    in1 = [{"k": Ks[i], "v": Vs[i], "amu": consts["amu"], "aml": consts["aml"],
            "ident": consts["ident"], "xinit": xinit} for i in range(NCORE)]
    r1 = run_bass_kernel_spmd(nc1, in1, core_ids=list(range(NCORE)))

    # host: compose transitions -> per-core W_start
    W0d = W0.astype(np.float64)
    ws = [W0d]
    for i in range(NCORE - 1):
        pg = r1.results[i]["pg"].astype(np.float64)
        P, G = pg[:, :DK], pg[:, DK:]
        ws.append(G + P @ (ws[-1] - W0d))

    nc2 = _get(2, nch, NCORE)
    in2 = [{"k": Ks[i], "v": Vs[i], "q": Qs[i],
            "ta": r1.results[i]["ta_out"],
            "incl": consts["incl"], "ident": consts["ident"],
            "winit": ws[i].astype(np.float32)} for i in range(NCORE)]
    r2 = run_bass_kernel_spmd(nc2, in2, core_ids=list(range(NCORE)))
    global LAST_RESULTS
    LAST_RESULTS = (r1, r2)

    out = np.empty((Bs, Ss, DV), np.float32)
    for i in range(NCORE):
        yi = r2.results[i]["y"].reshape(spc, Bs, DV).transpose(1, 0, 2)
        out[:, i * spc:(i + 1) * spc, :] = yi
    return out
